# revision 4
# baseline (speedup 1.0000x reference)
"""Group-limited MoE router kernel for Trainium2 (Bass/Tile), 8-core SPMD.

Per token (row of 256 experts):
  scores = sigmoid(logits); biased = scores + bias
  group_score[g] = top2sum(biased[g*32:(g+1)*32]) for 8 groups of 32
  keep top-4 groups; topk_ids = top-8 of masked biased (descending)
  weights = scores[topk_ids], renormalized to sum 1, * 2.5

Data-parallel over tokens: 131072 tokens -> 8 cores x 16384.
Tokens on SBUF partitions, experts on the free dim; elementwise work is
batched 8 slabs (1024 tokens) per instruction.

Algorithm (payload-packed ranking, ~3.5x faster than the naive 12-pass
top-k pipeline):

1. u2 = fl(scores + (bias + 192)). fp32 ulp at 192 is 2^-16, so this one
   add rounds biased onto a 2^-16 grid. v = u2 - 192 is exact (Sterbenz).
2. packed = scores * 2^-17 + v: the winner's score rides in the low mantissa
   bits strictly below the rank grid. Ranking by packed == ranking by biased
   up to ~2^-16 near-ties (measured 675/1M flipped ids, total rel err 2.8e-3
   on the reference distribution), and one max8 scan yields BOTH the top-8
   order and the weights: pay = vals8 - round_grid(vals8) = score * 2^-17.
   This eliminates the second top-8 pass, the index-association pass, and
   the gather of scores at the winning ids.
3. Group top-2 sums: segmented max-reduce (m1), match_replace to knock out
   each group's max, second segmented max-reduce (m2); gs = m1 + m2.
4. Top-4 groups per slab via one max8 on the 8 group scores; losers get
   -4096 added; one max8 + max_index over the masked row gives ids.

Engine split (TRN2 ISA-legal): Act: sigmoid, v=u2-192, payload rounding.
Pool: u2, mask add, small mults. DVE: reduces, match_replace, max8,
max_index, packed, comparisons, reciprocal. Software-pipelined with a
1-batch skew between the ranking stage and the top-8 stage.
"""

import numpy as np

TOKENS = 131072
E = 256
G = 8
EPG = 32
K = 8
SCALE = 2.5
N_CORES = 8
TPC = TOKENS // N_CORES

OFF = 192.0  # grid offset: ulp(192) = 2^-16
PAYS = float(2.0**-17)  # score payload scale, strictly below the grid
NEGBIG = -4096.0


def build_kernel(tpc: int, reps: int = 1):
    import concourse.bass as bass
    import concourse.bacc as bacc
    import concourse.mybir as mybir
    from concourse.tile import TileContext

    f32 = mybir.dt.float32
    u32 = mybir.dt.uint32

    nc = bacc.Bacc()
    logits_d = nc.declare_dram_parameter("logits", [tpc, E], f32, isOutput=False)
    b2_d = nc.declare_dram_parameter("bias", [1, E], f32, isOutput=False)
    w_d = nc.declare_dram_parameter("weights", [tpc, K], f32, isOutput=True)
    i_d = nc.declare_dram_parameter("ids", [tpc, K], u32, isOutput=True)

    P = 128
    S = 8
    TB = P * S
    SE = S * E
    n_batch = tpc // TB
    assert n_batch * TB == tpc

    Sigmoid = mybir.ActivationFunctionType.Sigmoid
    Copy = mybir.ActivationFunctionType.Copy
    Alu = mybir.AluOpType
    AxX = mybir.AxisListType.X

    with TileContext(nc) as tc:
        with (
            tc.tile_pool(name="const", bufs=1) as const_pool,
            tc.tile_pool(name="big", bufs=3) as big,
            tc.tile_pool(name="small", bufs=6) as small,
            tc.tile_pool(name="outp", bufs=4) as outp,
        ):
            b2_sb = const_pool.tile([P, E], f32)
            nc.sync.dma_start(out=b2_sb, in_=b2_d[:].to_broadcast([P, E]))
            b2_bc = b2_sb.unsqueeze(1).to_broadcast([P, S, E])

            def stage_front(b):
                t0 = b * TB
                src = logits_d[t0 : t0 + TB, :].rearrange("(s p) e -> p s e", p=P)
                x = big.tile([P, S, E], f32, tag="x")
                nc.sync.dma_start(out=x, in_=src)
                scores = big.tile([P, S, E], f32, tag="scores")
                nc.scalar.activation(out=scores, in_=x, func=Sigmoid)

                u2 = big.tile([P, S, E], f32, tag="u2")
                nc.gpsimd.tensor_tensor(out=u2, in0=scores, in1=b2_bc, op=Alu.add)

                v = big.tile([P, S, E], f32, tag="v")
                nc.scalar.activation(out=v, in_=u2, func=Copy, bias=-OFF)

                # group top-2 sums: m1 reduce, knock out each group's max via
                # match_replace, m2 reduce. (A tensor_tensor_scan pair-best
                # formulation sims faster but the serial scan runs ~2-4x
                # slower than modeled on real silicon - measured 351us vs
                # 202us/iter for this version.)
                u2g = u2.rearrange("p s (g e) -> p s g e", g=G)
                m1 = small.tile([P, S, G], f32, tag="m1")
                nc.vector.tensor_reduce(out=m1, in_=u2g, axis=AxX, op=Alu.max)
                rep = big.tile([P, S, E], f32, tag="r")
                for s in range(S):
                    nc.vector.match_replace(
                        out=rep[:, s], in_to_replace=m1[:, s],
                        in_values=u2[:, s], imm_value=NEGBIG,
                    )
                m2 = small.tile([P, S, G], f32, tag="m2")
                nc.vector.tensor_reduce(
                    out=m2, in_=rep.rearrange("p s (g e) -> p s g e", g=G),
                    axis=AxX, op=Alu.max,
                )
                gs = small.tile([P, S, G], f32, tag="gs")
                nc.gpsimd.tensor_tensor(out=gs, in0=m1, in1=m2, op=Alu.add)
                packed = big.tile([P, S, E], f32, tag="packed")
                nc.vector.scalar_tensor_tensor(
                    out=packed, in0=scores, scalar=PAYS, in1=v,
                    op0=Alu.mult, op1=Alu.add,
                )
                pg = packed.rearrange("p s (g e) -> p s g e", g=G)

                g8 = small.tile([P, S, 8], f32, tag="g8")
                for s in range(S):
                    nc.vector.max(out=g8[:, s], in_=gs[:, s])
                thr = g8[:, :, 3:4].to_broadcast([P, S, G])
                neg = small.tile([P, S, G], f32, tag="neg")
                nc.vector.tensor_tensor(out=neg, in0=gs, in1=thr, op=Alu.is_lt)
                nc.gpsimd.tensor_scalar(
                    out=neg, in0=neg, scalar1=NEGBIG, scalar2=None, op0=Alu.mult
                )
                negb = neg.unsqueeze(3).to_broadcast([P, S, G, EPG])
                nc.gpsimd.tensor_tensor(out=pg, in0=pg, in1=negb, op=Alu.add)
                return packed, b

            def stage_back(state):
                packed, b = state
                t0 = b * TB
                v8 = small.tile([P, S, K], f32, tag="v8")
                i8 = outp.tile([P, S, K], u32, tag="i8")
                for s in range(S):
                    nc.vector.max(out=v8[:, s], in_=packed[:, s])
                    nc.vector.max_index(
                        out=i8[:, s], in_max=v8[:, s], in_values=packed[:, s]
                    )
                q1 = small.tile([P, S, K], f32, tag="q1")
                nc.scalar.activation(out=q1, in_=v8, func=Copy, bias=OFF)
                nc.scalar.activation(out=q1, in_=q1, func=Copy, bias=-OFF)
                pay = small.tile([P, S, K], f32, tag="pay")
                nc.gpsimd.tensor_tensor(out=pay, in0=v8, in1=q1, op=Alu.subtract)
                wsum = small.tile([P, S, 1], f32, tag="wsum")
                nc.vector.tensor_reduce(out=wsum, in_=pay, axis=AxX, op=Alu.add)
                nc.vector.tensor_scalar(
                    out=wsum, in0=wsum, scalar1=1.0 / SCALE, scalar2=None,
                    op0=Alu.mult,
                )
                rcp = small.tile([P, S, 1], f32, tag="rcp")
                nc.vector.reciprocal(out=rcp, in_=wsum)
                wout = outp.tile([P, S, K], f32, tag="wout")
                nc.gpsimd.tensor_tensor(
                    out=wout, in0=pay, in1=rcp.to_broadcast([P, S, K]), op=Alu.mult
                )
                wdst = w_d[t0 : t0 + TB, :].rearrange("(s p) k -> p s k", p=P)
                idst = i_d[t0 : t0 + TB, :].rearrange("(s p) k -> p s k", p=P)
                nc.scalar.dma_start(out=wdst, in_=wout)
                nc.scalar.dma_start(out=idst, in_=i8)

            def whole_pass():
                pending = None
                for b in range(n_batch):
                    st = stage_front(b)
                    if pending is not None:
                        stage_back(pending)
                    pending = st
                stage_back(pending)

            if reps == 1:
                whole_pass()
            else:
                with tc.For_i(0, reps, 1):
                    whole_pass()

    nc.finalize()
    return nc


def build_kernel_rep(tpc: int, reps: int):
    return build_kernel(tpc, reps=reps)


_NC_CACHE = {}


def _get_nc(tpc: int):
    if tpc not in _NC_CACHE:
        _NC_CACHE[tpc] = build_kernel(tpc)
    return _NC_CACHE[tpc]


def make_in_maps(router_logits: np.ndarray, expert_bias: np.ndarray):
    tokens = router_logits.shape[0]
    tpc = tokens // N_CORES
    b2 = (expert_bias.astype(np.float32) + np.float32(OFF)).reshape(1, E)
    return [
        {
            "logits": np.ascontiguousarray(router_logits[c * tpc : (c + 1) * tpc]),
            "bias": b2,
        }
        for c in range(N_CORES)
    ]


def kernel(router_logits: np.ndarray, expert_bias: np.ndarray, _trace: bool = False):
    from concourse.bass_utils import run_bass_kernel_spmd

    router_logits = np.asarray(router_logits, dtype=np.float32)
    expert_bias = np.asarray(expert_bias, dtype=np.float32)
    tokens = router_logits.shape[0]
    assert tokens % N_CORES == 0
    tpc = tokens // N_CORES

    nc = _get_nc(tpc)
    in_maps = make_in_maps(router_logits, expert_bias)
    res = run_bass_kernel_spmd(
        nc, in_maps, core_ids=list(range(N_CORES)), trace=_trace
    )
    weights = np.concatenate([r["weights"] for r in res.results], axis=0)
    ids = np.concatenate([r["ids"] for r in res.results], axis=0).astype(np.int32)
    if _trace:
        kernel.last_exec_time_ns = res.exec_time_ns
        kernel.last_mean_exec_time_ns = res.mean_exec_time_ns
    return weights, ids
